# revision 4
# baseline (speedup 1.0000x reference)
"""DeepFM-style (fm1 + fm2 + DNN) Trainium2 kernel, batch-sharded across 8 NeuronCores.

Strategy
--------
Each core handles 2048 batch rows with a full replica of the (merged fm1+fm2)
embedding table, so no collectives are needed.  The random-access embedding
lookup is the bottleneck (memory regime): SWDGE descriptor generation costs
~8ns/row and only dma_gather amortizes it across 4 parallel SWDGE queues.
dma_gather needs int16 indices, so the gather runs in two phases per
512-row batch quarter:

  phase 1  HBM -> SBUF : rows bucketed into 80 static 32767-row windows of the
           flat [26*100000, 128]-bf16 table; one compact dma_gather per window
           (count supplied per-core via a register), landing window-sorted in a
           [128, 160, 128] staging tile (position i -> partition i%128, col i//128).
  phase 2  SBUF -> SBUF: per feature f, a transposed SBUF-source dma_gather
           un-sorts by rank (host-computed position of each (b,f) entry) and
           lands feature-major X_f [128 dims, 512 batch] directly.

Table rows place each f's 16 embedding dims + fm1 scalar at partition slot
(f%7)*18, so 7 features pack into one 128-partition K-chunk with a plain DVE
add; the whole model is then PSUM-accumulated matmuls.
"""

import numpy as np
import ml_dtypes

from contextlib import ExitStack

import concourse.tile as tile
from concourse import bass, mybir, bacc
from concourse.library_config import mlp
from concourse.bass_utils import run_bass_kernel_spmd

# model dims
B, D_DENSE, F, V, ED, REP = 16384, 13, 26, 100000, 16, 64
H1, H2 = 256, 128
NCORES = 8
BL = B // NCORES            # 2048 local batch rows per core
NQ = 4                      # batch quarters per core
QB = BL // NQ               # 512 rows per quarter
FV = F * V                  # 2.6M table rows
ROWW = 128                  # bf16 elements per table row (256 B)
WROWS = 32767               # int16-addressable window size
NWIN = -(-FV // WROWS)      # 80 windows
CAP = 240                   # idx budget per (quarter, window); multiple of 16
CPW = -(-CAP // 128)        # staging columns per window (2)
SCOLS = NWIN * CPW          # staging columns (160)
FPC = 7                     # features per K-chunk
SLOTW = 18                  # partition slot width: 16 dims + fm1 + pad
NCHUNK = 4                  # ceil(26/7)
KDENSE = D_DENSE + 1 + REP  # 78: dense_x rows + ones row + rep rows

BF16 = mybir.dt.bfloat16
F32 = mybir.dt.float32
I16 = mybir.dt.int16
I32 = mybir.dt.int32


def _split_waits(nc, max_waits=1):
    """walrus here encodes at most one sync wait per instruction; hoist extras
    onto InstEventSemaphore carriers on the same engine."""
    for f in nc.m.functions:
        for bb in f.blocks:
            new_insts = []
            for inst in bb.instructions:
                si = inst.sync_info
                if si and si.on_wait and len(si.on_wait) > max_waits:
                    waits = list(si.on_wait)
                    for i, w in enumerate(waits[:-max_waits]):
                        ev = mybir.InstEventSemaphore(
                            name=f"{inst.name}-waitsplit{i}", ins=[], outs=[])
                        ev.engine = inst.engine
                        ev.sync_info = mybir.SyncInfo(on_wait=[w], on_update=[])
                        new_insts.append(ev)
                    si.on_wait = waits[-max_waits:]
                new_insts.append(inst)
            bb.instructions[:] = new_insts


def _build_program():
    nc = bacc.Bacc("TRN2", num_swdge_queues=4)

    table = nc.declare_dram_parameter("table", [FV, ROWW], BF16, isOutput=False)
    ph1_idx = nc.declare_dram_parameter(
        "ph1_idx", [128, NQ * NWIN * (CAP // 16)], I16, isOutput=False)
    ph1_cnt = nc.declare_dram_parameter("ph1_cnt", [1, NQ * NWIN], I32, isOutput=False)
    ph2_idx = nc.declare_dram_parameter(
        "ph2_idx", [128, NQ * F * (QB // 16)], I16, isOutput=False)
    densrep = nc.declare_dram_parameter("densrep", [KDENSE, BL], F32, isOutput=False)
    w1c = nc.declare_dram_parameter("w1c", [NCHUNK * 128, H1], BF16, isOutput=False)
    w1d = nc.declare_dram_parameter("w1d", [KDENSE, H1], F32, isOutput=False)
    fm1w = nc.declare_dram_parameter("fm1w", [KDENSE, 1], F32, isOutput=False)
    sel = nc.declare_dram_parameter("sel", [128, 17], BF16, isOutput=False)
    negmask = nc.declare_dram_parameter("negmask", [128, 1], BF16, isOutput=False)
    halves16 = nc.declare_dram_parameter("halves16", [16, 1], F32, isOutput=False)
    e17 = nc.declare_dram_parameter("e17", [17, 1], F32, isOutput=False)
    w2 = nc.declare_dram_parameter("w2", [H1, H2], BF16, isOutput=False)
    b2row = nc.declare_dram_parameter("b2row", [1, H2], BF16, isOutput=False)
    fw = nc.declare_dram_parameter("fw", [H2, 1], BF16, isOutput=False)
    out = nc.declare_dram_parameter("out", [1, BL], F32, isOutput=True)

    with tile.TileContext(nc) as tc, ExitStack() as ctx:
        cpool = ctx.enter_context(tc.tile_pool(name="const", bufs=1))
        stgpool = ctx.enter_context(tc.tile_pool(name="stg", bufs=1))
        xfpool = ctx.enter_context(tc.tile_pool(name="xf", bufs=1))
        xcpool = ctx.enter_context(tc.tile_pool(name="xc", bufs=2))
        hpool = ctx.enter_context(tc.tile_pool(name="h", bufs=2))
        spool = ctx.enter_context(tc.tile_pool(name="scratch", bufs=2))
        ppool = ctx.enter_context(tc.tile_pool(name="psum", bufs=1, space="PSUM"))
        ppool2 = ctx.enter_context(tc.tile_pool(name="psum2", bufs=2, space="PSUM"))

        nc.gpsimd.load_library(mlp)

        # constants / weights into SBUF
        ph1_idx_t = cpool.tile([128, NQ * NWIN * (CAP // 16)], I16)
        nc.sync.dma_start(out=ph1_idx_t[:], in_=ph1_idx[:])
        ph1_cnt_t = cpool.tile([1, NQ * NWIN], I32)
        nc.sync.dma_start(out=ph1_cnt_t[:], in_=ph1_cnt[:])
        ph2_idx_t = cpool.tile([128, NQ * F * (QB // 16)], I16)
        nc.sync.dma_start(out=ph2_idx_t[:], in_=ph2_idx[:])
        densrep_t = cpool.tile([KDENSE, BL], F32)
        nc.sync.dma_start(out=densrep_t[:], in_=densrep[:])
        w1c_t = []
        for g in range(NCHUNK):
            t = cpool.tile([128, H1], BF16, tag=f"w1c{g}")
            nc.sync.dma_start(out=t[:], in_=w1c[g * 128:(g + 1) * 128, :])
            w1c_t.append(t)
        w1d_t = cpool.tile([KDENSE, H1], F32)
        nc.sync.dma_start(out=w1d_t[:], in_=w1d[:])
        fm1w_t = cpool.tile([KDENSE, 1], F32)
        nc.sync.dma_start(out=fm1w_t[:], in_=fm1w[:])
        sel_t = cpool.tile([128, 17], BF16)
        nc.sync.dma_start(out=sel_t[:], in_=sel[:])
        negmask_t = cpool.tile([128, 1], BF16)
        nc.sync.dma_start(out=negmask_t[:], in_=negmask[:])
        halves16_t = cpool.tile([16, 1], F32)
        nc.sync.dma_start(out=halves16_t[:], in_=halves16[:])
        e17_t = cpool.tile([17, 1], F32)
        nc.sync.dma_start(out=e17_t[:], in_=e17[:])
        w2_t = []
        for g in range(2):
            t = cpool.tile([128, H2], BF16, tag=f"w2{g}")
            nc.sync.dma_start(out=t[:], in_=w2[g * 128:(g + 1) * 128, :])
            w2_t.append(t)
        b2row_t = cpool.tile([1, H2], BF16)
        nc.sync.dma_start(out=b2row_t[:], in_=b2row[:])
        fw_t = cpool.tile([H2, 1], BF16)
        nc.sync.dma_start(out=fw_t[:], in_=fw[:])
        onesbf = cpool.tile([1, QB], BF16)
        nc.vector.memset(onesbf[:], 1.0)
        out_all = cpool.tile([1, BL], F32)

        regs = [ctx.enter_context(nc.gpsimd.register(f"cnt{i}")) for i in range(8)]

        for q in range(NQ):
            # ---- phase 1: windowed compact gather HBM -> staging ----
            stg = stgpool.tile([128, SCOLS, ROWW], BF16, tag="stg")
            for w in range(NWIN):
                k = q * NWIN + w
                base = w * WROWS
                nrows = min(WROWS, FV - base)
                reg = regs[w % 8]
                nc.gpsimd.reg_load(reg, ph1_cnt_t[0:1, k:k + 1])
                nc.gpsimd.dma_gather(
                    out_ap=stg[:, w * CPW:(w + 1) * CPW, :],
                    in_ap=table[base:base + nrows, :],
                    idxs_ap=ph1_idx_t[:, k * (CAP // 16):(k + 1) * (CAP // 16)],
                    num_idxs=CAP,
                    num_idxs_reg=reg,
                    elem_size=ROWW,
                    single_packet=True,
                    queue_num=w % 4,
                )
            # ---- phase 2: SBUF-source transposed un-sort gather -> X_f ----
            xf = []
            for f in range(F):
                t = xfpool.tile([128, 1, QB], BF16, tag=f"xf{f}")
                j = q * F + f
                nc.gpsimd.dma_gather(
                    out_ap=t[:],
                    in_ap=stg[:],
                    idxs_ap=ph2_idx_t[:, j * (QB // 16):(j + 1) * (QB // 16)],
                    num_idxs=QB,
                    num_idxs_reg=QB,
                    elem_size=ROWW,
                    transpose=True,
                    sbuf_tokens_per_rank=128,
                    sbuf_free_dim_per_rank=ROWW * 2,
                    sbuf_free_dim_pad_per_rank=0,
                    sbuf_byte_offset=0,
                    queue_num=f % 4,
                )
                xf.append(t)

            # ---- pack 7 features per 128-partition K-chunk (slots via table layout) ----
            xc = []
            for g in range(NCHUNK):
                t = xcpool.tile([128, QB], BF16, tag=f"xc{g}")
                fs = list(range(g * FPC, min((g + 1) * FPC, F)))
                nc.vector.tensor_add(
                    out=t[:], in0=xf[fs[0]][:, 0, :], in1=xf[fs[1]][:, 0, :])
                for f in fs[2:]:
                    nc.vector.tensor_add(out=t[:], in0=t[:], in1=xf[f][:, 0, :])
                xc.append(t)

            dr_c = densrep_t[:, q * QB:(q + 1) * QB]

            # ---- layer 1 (h1 = relu(W1^T X + b1)), feature-major ----
            h1p = []
            for h in range(2):
                p = ppool.tile([128, QB], F32, tag=f"h1p{h}")
                for g in range(NCHUNK):
                    nc.tensor.matmul(
                        out=p[:], lhsT=w1c_t[g][:, h * 128:(h + 1) * 128],
                        rhs=xc[g][:], start=(g == 0), stop=False)
                nc.tensor.matmul(
                    out=p[:], lhsT=w1d_t[:, h * 128:(h + 1) * 128], rhs=dr_c,
                    start=False, stop=True)
                h1p.append(p)
            h1sb = []
            for h in range(2):
                t = hpool.tile([128, QB], BF16, tag=f"h1sb{h}")
                nc.scalar.activation(
                    out=t[:], in_=h1p[h][:], func=mybir.ActivationFunctionType.Relu)
                h1sb.append(t)

            # ---- se / fm1 extraction: sepsum[0:16]=sum_f e, [16]=sum_f fm1 ----
            sepsum = ppool.tile([17, QB], F32, tag="sepsum")
            for g in range(NCHUNK):
                nc.tensor.matmul(out=sepsum[:], lhsT=sel_t[:], rhs=xc[g][:],
                                 start=(g == 0), stop=(g == NCHUNK - 1))
            se_sb = spool.tile([17, QB], F32, tag="se_sb")
            nc.vector.tensor_copy(out=se_sb[:], in_=sepsum[:])
            se2_sb = spool.tile([16, QB], F32, tag="se2_sb")
            nc.vector.tensor_mul(out=se2_sb[:], in0=se_sb[0:16, :], in1=se_sb[0:16, :])

            # ---- squared chunks for -0.5*sum(e^2) ----
            x2 = []
            for g in range(NCHUNK):
                t = xcpool.tile([128, QB], BF16, tag=f"x2{g}")
                nc.vector.tensor_mul(out=t[:], in0=xc[g][:], in1=xc[g][:])
                x2.append(t)

            # ---- layer 2 + final, and the big accumulation into out_psum ----
            h2p = ppool.tile([128, QB], F32, tag="h2p")
            nc.tensor.matmul(out=h2p[:], lhsT=w2_t[0][:], rhs=h1sb[0][:],
                             start=True, stop=False)
            nc.tensor.matmul(out=h2p[:], lhsT=w2_t[1][:], rhs=h1sb[1][:],
                             start=False, stop=False)
            nc.tensor.matmul(out=h2p[:], lhsT=b2row_t[:], rhs=onesbf[:],
                             start=False, stop=True)
            h2sb = hpool.tile([128, QB], BF16, tag="h2sb")
            nc.scalar.activation(
                out=h2sb[:], in_=h2p[:], func=mybir.ActivationFunctionType.Relu)

            op = ppool2.tile([1, QB], F32, tag="outp")
            for g in range(NCHUNK):                         # -0.5 * sum e^2
                nc.tensor.matmul(out=op[:], lhsT=negmask_t[:], rhs=x2[g][:],
                                 start=(g == 0), stop=False)
            nc.tensor.matmul(out=op[:], lhsT=halves16_t[:], rhs=se2_sb[:],
                             start=False, stop=False)       # +0.5*sum se^2
            nc.tensor.matmul(out=op[:], lhsT=e17_t[:], rhs=se_sb[:],
                             start=False, stop=False)       # + fm1 sparse
            nc.tensor.matmul(out=op[:], lhsT=fm1w_t[:], rhs=dr_c,
                             start=False, stop=False)       # + fm1 dense + biases
            nc.tensor.matmul(out=op[:], lhsT=fw_t[:], rhs=h2sb[:],
                             start=False, stop=True)        # + dnn out
            nc.vector.tensor_copy(out=out_all[0:1, q * QB:(q + 1) * QB], in_=op[:])

        nc.sync.dma_start(out=out[:], in_=out_all[:])

    nc.compile()
    _split_waits(nc)
    return nc


_PROGRAM_CACHE = {}


def _get_program():
    if "nc" not in _PROGRAM_CACHE:
        _PROGRAM_CACHE["nc"] = _build_program()
    return _PROGRAM_CACHE["nc"]


def _prep_shared(fm1_tables, fm2_tables, fm1_dense_w, fm1_dense_b,
                 dnn_w1, dnn_b1, dnn_w2, dnn_b2, final_w, final_b):
    bf16 = ml_dtypes.bfloat16
    # merged table: row f*V+id holds e-dims at slot (f % FPC)*SLOTW, fm1 at +16
    tab = np.zeros((FV, ROWW), dtype=bf16)
    fm2 = np.ascontiguousarray(fm2_tables, dtype=np.float32).reshape(F, V, ED)
    fm1 = np.ascontiguousarray(fm1_tables, dtype=np.float32).reshape(F, V)
    for f in range(F):
        s = (f % FPC) * SLOTW
        tab[f * V:(f + 1) * V, s:s + ED] = fm2[f].astype(bf16)
        tab[f * V:(f + 1) * V, s + ED] = fm1[f].astype(bf16)

    # W1 chunks: chunk g row s*SLOTW+d  <->  w1 row (g*FPC+s)*ED+d
    w1 = np.asarray(dnn_w1, dtype=np.float32)
    w1c = np.zeros((NCHUNK * 128, H1), dtype=bf16)
    for g in range(NCHUNK):
        for s in range(min(FPC, F - g * FPC)):
            f = g * FPC + s
            w1c[g * 128 + s * SLOTW:g * 128 + s * SLOTW + ED, :] = \
                w1[f * ED:(f + 1) * ED, :].astype(bf16)

    # dense K-chunk rows: [dense_x(13) | ones(1) | rep(64)]
    w1d = np.zeros((KDENSE, H1), dtype=np.float32)
    w1d[0:D_DENSE, :] = w1[F * ED:F * ED + D_DENSE, :]
    w1d[D_DENSE, :] = np.asarray(dnn_b1, dtype=np.float32)
    w1d[D_DENSE + 1:, :] = w1[F * ED + D_DENSE:, :]

    fm1w = np.zeros((KDENSE, 1), dtype=np.float32)
    fdw = np.asarray(fm1_dense_w, dtype=np.float32).reshape(-1)
    fm1w[0:D_DENSE, 0] = fdw[0:D_DENSE]
    fm1w[D_DENSE, 0] = float(np.asarray(fm1_dense_b).reshape(-1)[0]) + \
        float(np.asarray(final_b).reshape(-1)[0])
    fm1w[D_DENSE + 1:, 0] = fdw[D_DENSE:]

    sel = np.zeros((128, 17), dtype=bf16)
    for k in range(128):
        r = k % SLOTW
        if r < ED:
            sel[k, r] = 1.0
        elif r == ED:
            sel[k, 16] = 1.0
    negmask = np.zeros((128, 1), dtype=bf16)
    for k in range(128):
        if k % SLOTW < ED:
            negmask[k, 0] = -0.5
    halves16 = np.full((16, 1), 0.5, dtype=np.float32)
    e17 = np.zeros((17, 1), dtype=np.float32)
    e17[16, 0] = 1.0

    return dict(
        table=tab, w1c=w1c, w1d=w1d, fm1w=fm1w, sel=sel, negmask=negmask,
        halves16=halves16, e17=e17,
        w2=np.asarray(dnn_w2, dtype=np.float32).astype(bf16),
        b2row=np.asarray(dnn_b2, dtype=np.float32).reshape(1, H2).astype(bf16),
        fw=np.asarray(final_w, dtype=np.float32).reshape(H2, 1).astype(bf16),
    )


def _prep_core(sparse_ids, dense_x, representation):
    """Per-core index/layout prep: phase-1 window buckets + phase-2 ranks."""
    ids = np.asarray(sparse_ids, dtype=np.int64)              # [BL, F]
    flat = ids + (np.arange(F, dtype=np.int64) * V)[None, :]  # [BL, F]
    w_e = (flat // WROWS).astype(np.int64)                    # window per entry
    lidx = (flat - w_e * WROWS).astype(np.int16)              # idx within window
    q_e = (np.arange(BL, dtype=np.int64) // QB)[:, None].repeat(F, axis=1)

    ph1_idx = np.full((NQ * NWIN, CAP), -1, dtype=np.int16)
    ph1_cnt = np.zeros((1, NQ * NWIN), dtype=np.int32)
    rank = np.zeros((BL, F), dtype=np.int32)

    qf = q_e.reshape(-1)
    wf = w_e.reshape(-1)
    lf = lidx.reshape(-1)
    # order entries by (quarter, window), stable in (b, f)
    order = np.lexsort((wf, qf))
    qs, ws, ls = qf[order], wf[order], lf[order]
    key = qs * NWIN + ws
    # j = position within its (q, w) bucket
    uniq, first_pos, counts = np.unique(key, return_index=True, return_counts=True)
    if counts.max() > CAP:
        raise RuntimeError(f"window bucket overflow: {counts.max()} > CAP={CAP}")
    j = np.arange(len(key)) - np.repeat(first_pos, counts)
    ph1_idx[key, j] = ls
    ph1_cnt[0, uniq] = counts
    # rank (phase-2 idx) = window column base * 128 + j
    rnk = (ws * (CPW * 128) + j).astype(np.int32)
    rank.reshape(-1)[order] = rnk

    # wrap phase-1 idx: [NQ*NWIN, CAP] -> per instr [16, CAP/16] -> [128, .]
    w1 = ph1_idx.reshape(NQ * NWIN, CAP // 16, 16).transpose(0, 2, 1)  # [., 16, CAP/16]
    ph1_wrapped = np.tile(w1, (1, 8, 1)).transpose(1, 0, 2).reshape(
        128, NQ * NWIN * (CAP // 16))

    # phase-2 idx: per (q, f): ranks of (b = q*QB + j2, f), wrapped
    r = rank.reshape(NQ, QB, F).transpose(0, 2, 1).astype(np.int16)  # [NQ, F, QB]
    w2_ = r.reshape(NQ * F, QB // 16, 16).transpose(0, 2, 1)          # [., 16, QB/16]
    ph2_wrapped = np.tile(w2_, (1, 8, 1)).transpose(1, 0, 2).reshape(
        128, NQ * F * (QB // 16))

    densrep = np.empty((KDENSE, BL), dtype=np.float32)
    densrep[0:D_DENSE] = np.asarray(dense_x, dtype=np.float32).T
    densrep[D_DENSE] = 1.0
    densrep[D_DENSE + 1:] = np.asarray(representation, dtype=np.float32).T

    return dict(ph1_idx=ph1_wrapped, ph1_cnt=ph1_cnt, ph2_idx=ph2_wrapped,
                densrep=densrep)


def kernel(representation, dense_x, sparse_ids, fm1_tables, fm2_tables,
           fm1_dense_w, fm1_dense_b, dnn_w1, dnn_b1, dnn_w2, dnn_b2,
           final_w, final_b):
    nc = _get_program()
    shared = _prep_shared(fm1_tables, fm2_tables, fm1_dense_w, fm1_dense_b,
                          dnn_w1, dnn_b1, dnn_w2, dnn_b2, final_w, final_b)
    in_maps = []
    for i in range(NCORES):
        sl = slice(i * BL, (i + 1) * BL)
        core = _prep_core(np.asarray(sparse_ids)[sl],
                          np.asarray(dense_x)[sl],
                          np.asarray(representation)[sl])
        in_maps.append({**shared, **core})
    res = run_bass_kernel_spmd(nc, in_maps, core_ids=list(range(NCORES)))
    out = np.concatenate(
        [res.results[i]["out"].reshape(-1) for i in range(NCORES)])
    return out.reshape(B, 1).astype(np.float32)
